# revision 1
# baseline (speedup 1.0000x reference)
"""Luong attention energies + softmax on 8 TRN2 NeuronCores.

reference math (per core, batch-sharded):
  energy[b,s] = <hid[b], enc[s,b]> + (hid[b] @ A) . emb[s,b]
  out[b,0,s]  = softmax_s(energy[b,s])

Full shapes: hidden [1,64,512] f32, encoder_outputs [2048,64,512] f32,
embedding [2048,64,3] f32, affect_matrix [512,3] f32 -> out [64,1,2048] f32.

Sharding: batch dim 64 -> 8 cores x 8. No cross-core communication.

Per-core plan (memory-bound: 32 MB encoder shard, ~90 us at 358 GB/s):
  GpSimd elementwise is avoided: it shares an SBUF port with the DVE and
  stalls 2-port DVE streams almost 1:1. GpSimd only does broadcasts and
  the emba DMA (SWDGE, off the HWDGE rings the enc stream uses).
  stream enc in 16 per-tile DMAs (2 MB each, bufs=6 lookahead):
    DVE : one grouped mult per tile (all 8 b) -> pd, reduce b0-1
          (+ b2 on even tiles)
    ACT : Copy-with-accum reduces b3-7 (+ b2 on odd tiles), junk out in
          PSUM (ScalarE is closer to PSUM)
  the affect-term chain runs in the DMA shadow of tiles 0-1 so no engine
  queue blocks mid-stream.
  epilogue without the true max: exp(e/2-25) on ACT then squared on DVE
  (= exp(e-50), f32-safe); PE ones-matmul column sums; DVE reciprocal;
  PE transpose puts (b,t) on partitions so the 1/sum is a per-partition
  ACT scale fused into the PSUM->SBUF copy; direct strided store.
"""

import numpy as np

S, B, H, E = 2048, 64, 512, 3
N_CORES = 8
BS = B // N_CORES      # 8 batches per core
NT = S // 128          # 16 s-tiles of 128 rows

_CACHE = {}


def _build_nc():
    import concourse.bass as bass
    import concourse.tile as tile
    from concourse import bacc, mybir
    from concourse.mybir import AluOpType as alu
    from concourse.mybir import ActivationFunctionType as actf

    f32 = mybir.dt.float32

    nc = bacc.Bacc("TRN2", target_bir_lowering=False, debug=False)
    enc = nc.dram_tensor("enc", [S, BS, H], f32, kind="ExternalInput").ap()
    emb = nc.dram_tensor("emb", [S, BS, E], f32, kind="ExternalInput").ap()
    hid = nc.dram_tensor("hid", [1, BS, H], f32, kind="ExternalInput").ap()
    amat = nc.dram_tensor("amat", [H, E], f32, kind="ExternalInput").ap()
    out = nc.dram_tensor("out", [BS, 1, S], f32, kind="ExternalOutput").ap()

    with tile.TileContext(nc) as tc:
        with (
            tc.tile_pool(name="persist", bufs=1) as pp,
            tc.tile_pool(name="enc", bufs=6) as encp,
            tc.tile_pool(name="pd", bufs=4) as pdp,
            tc.tile_pool(name="pjunk", bufs=1, space="PSUM") as pjp,
            tc.tile_pool(name="psum", bufs=2, space="PSUM") as psp,
            tc.tile_pool(name="pbc", bufs=4, space="PSUM") as pbcp,
        ):
            # ---- energy tiles: P col = b*NT + t ----
            EbufD = pp.tile([128, 2 * NT], f32)   # b0..1 (DVE)
            EbufM = pp.tile([128, NT], f32)       # b2 (DVE t%4==0 / ACT else)
            EbufA = pp.tile([128, 5 * NT], f32)   # b3..7 (ACT)
            junkA = pjp.tile([128, H], f32)       # ACT accum main-out (PSUM)

            # ---- prologue: everything with no enc dependency, scheduled
            # before the stream so no engine queue blocks later ----
            hidrow = pp.tile([1, BS * H], f32)
            nc.sync.dma_start(hidrow[:], hid.rearrange("o b h -> o (b h)"))
            # hidden broadcast via K=1 PE matmuls (ones[1,128] x hidrow
            # chunk -> PSUM) drained by DVE/ACT copies while both are idle
            # waiting for the first enc tile. GpSimd compute here would sit
            # behind a SWDGE-drain fence until ~22 us.
            hid8 = pp.tile([BS, H], f32)
            nc.scalar.dma_start(hid8[:], hid[0])
            onesr = pp.tile([1, 128], f32)
            nc.vector.memset(onesr[:], 1.0)
            hidb = pp.tile([128, BS * H], f32)
            for c in range(8):
                pb = pbcp.tile([128, 512], f32, tag="pb")
                nc.tensor.matmul(pb[:], onesr[:],
                                 hidrow[0:1, c * 512:(c + 1) * 512])
                if c % 2 == 0:
                    nc.vector.tensor_copy(hidb[:, c * 512:(c + 1) * 512], pb[:])
                else:
                    nc.scalar.copy(hidb[:, c * 512:(c + 1) * 512], pb[:])
            hidb_v = hidb[:].rearrange("p (b h) -> p b h", h=H)
            ones1 = pp.tile([128, 1], f32)
            nc.vector.memset(ones1[:], 1.0)
            ebias = pp.tile([128, 1], f32)
            nc.vector.memset(ebias[:], -25.0)

            # ---- main loop: one DMA + compute per s-tile ----
            for t in range(NT):
                et = encp.tile([128, BS * H], f32, tag="et")
                et_v = et[:].rearrange("p (b h) -> p b h", h=H)
                nc.sync.dma_start(et_v, enc[t * 128:(t + 1) * 128])

                # per-tile compute
                pd = pdp.tile([128, BS * H], f32, tag="pd")
                pd_v = pd[:].rearrange("p (b h) -> p b h", h=H)
                nc.vector.tensor_tensor(pd_v, et_v, hidb_v, alu.mult)
                nc.vector.tensor_reduce(
                    EbufD[:].rearrange("p (b t) -> p b t", t=NT)[:, :, t:t + 1],
                    pd_v[:, 0:2, :],
                    axis=mybir.AxisListType.X, op=alu.add)
                if t % 4 == 0:   # b2 reduce: 1 in 4 tiles on DVE, rest ACT
                    nc.vector.tensor_reduce(
                        EbufM[:, t:t + 1], pd_v[:, 2, :],
                        axis=mybir.AxisListType.X, op=alu.add)
                else:
                    nc.scalar.activation(
                        junkA[:], pd_v[:, 2, :], actf.Copy,
                        accum_out=EbufM[:, t:t + 1])
                for b in range(3, BS):
                    nc.scalar.activation(
                        junkA[:], pd_v[:, b, :], actf.Copy,
                        accum_out=EbufA[:, (b - 3) * NT + t:(b - 3) * NT + t + 1])

                if t == 1:
                    # SWDGE DMAs issued only now: their slow descriptor
                    # drain would otherwise fence the hidb broadcast
                    am128 = pp.tile([128, 4 * E], f32)
                    nc.gpsimd.dma_start(
                        am128[:].rearrange("p (c e) -> p c e", e=E),
                        amat.rearrange("(c p) e -> p c e", p=128))
                    # ---- identity matrix (iota IRAM loads happen here,
                    # after the hidb broadcast is long done) ----
                    pidx = pp.tile([128, 1], f32)
                    nc.gpsimd.iota(pidx[:], pattern=[[0, 1]], base=0,
                                   channel_multiplier=1,
                                   allow_small_or_imprecise_dtypes=True)
                    colidx = pp.tile([128, 128], f32)
                    nc.gpsimd.iota(colidx[:], pattern=[[1, 128]], base=0,
                                   channel_multiplier=0,
                                   allow_small_or_imprecise_dtypes=True)
                    ident = pp.tile([128, 128], f32)
                    nc.vector.tensor_scalar(ident[:], colidx[:], pidx[:, 0:1],
                                            None, alu.is_equal)
                    # ---- affect-term chain in the tile-2.. DMA shadow ----
                    emba = pp.tile([128, NT * BS * E], f32)
                    emba_v = emba[:].rearrange("p (t b e) -> p t b e", b=BS, e=E)
                    nc.gpsimd.dma_start(emba_v, emb.rearrange("(t p) b e -> p t b e", p=128))
                    # hA[b,e] = sum_h hid[b,h] * A[h,e] on the PE:
                    # transpose hid8 into [h', (c, b)] chunks, then 4
                    # accumulating [128,8]x[128,3] matmuls
                    hT_ps = psp.tile([128, 4 * BS], f32, tag="ps")
                    for c in range(4):
                        nc.tensor.transpose(
                            hT_ps[:, c * BS:(c + 1) * BS],
                            hid8[:, c * 128:(c + 1) * 128], ident[0:BS, 0:BS])
                    hT = pp.tile([128, 4 * BS], f32)
                    nc.vector.tensor_copy(hT[:], hT_ps[:])
                    hA_ps = psp.tile([BS, E], f32, tag="ps")
                    for c in range(4):
                        nc.tensor.matmul(
                            hA_ps[:], hT[:, c * BS:(c + 1) * BS],
                            am128[:].rearrange("p (c e) -> p c e", e=E)[:, c, :],
                            start=(c == 0), stop=(c == 3))
                    hA = pp.tile([BS, E], f32)
                    nc.vector.tensor_copy(hA[:], hA_ps[:])
                    harow = pp.tile([1, BS * E], f32)
                    nc.scalar.dma_start(harow[0:1].rearrange("o (b e) -> o b e", e=E), hA[:])
                    hab = pp.tile([128, BS * E], f32)
                    nc.gpsimd.partition_broadcast(hab[:], harow[0:1, :])

                    # aff[p, t, b] = sum_e emb[t*128+p, b, e] * hA[b, e]
                    afftmp = pp.tile([128, NT * BS * E], f32)
                    nc.vector.tensor_tensor(
                        afftmp[:].rearrange("p (t b e) -> p t b e", b=BS, e=E),
                        emba_v,
                        hab[:].rearrange("p (b e) -> p b e", e=E)
                        .unsqueeze(1).broadcast_to([128, NT, BS, E]),
                        alu.mult)
                    aff = pp.tile([128, NT * BS], f32)
                    aff_v = aff[:].rearrange("p (t b) -> p t b", b=BS)
                    nc.vector.tensor_reduce(
                        aff_v, afftmp[:].rearrange("p (t b e) -> p t b e", b=BS, e=E),
                        axis=mybir.AxisListType.X, op=alu.add)

            # ---- epilogue ----
            EbufD_v = EbufD[:].rearrange("p (b t) -> p b t", t=NT)
            EbufA_v = EbufA[:].rearrange("p (b t) -> p b t", t=NT)
            nc.vector.tensor_tensor(
                EbufD_v, EbufD_v, aff_v[:, :, 0:2].transpose([0, 2, 1]), alu.add)
            nc.vector.tensor_tensor(
                EbufM[:].unsqueeze(1), EbufM[:].unsqueeze(1),
                aff_v[:, :, 2:3].transpose([0, 2, 1]), alu.add)
            nc.vector.tensor_tensor(
                EbufA_v, EbufA_v, aff_v[:, :, 3:BS].transpose([0, 2, 1]), alu.add)

            # exp(e/2 - 25) then square = exp(e - 50), f32-safe
            P = pp.tile([128, 128], f32)
            nc.scalar.activation(P[:, 0:2 * NT], EbufD[:], actf.Exp,
                                 bias=ebias[:, 0:1], scale=0.5)
            nc.scalar.activation(P[:, 2 * NT:3 * NT], EbufM[:], actf.Exp,
                                 bias=ebias[:, 0:1], scale=0.5)
            nc.scalar.activation(P[:, 3 * NT:128], EbufA[:], actf.Exp,
                                 bias=ebias[:, 0:1], scale=0.5)
            nc.vector.tensor_tensor(P[:], P[:], P[:], alu.mult)

            # column sums over the 128 s-partitions: cs[0, b*16+t]
            cs = psp.tile([128, 128], f32, tag="ps")
            nc.tensor.matmul(cs[0:1, :], ones1[:], P[:])
            s8 = pp.tile([1, BS], f32)
            nc.vector.tensor_reduce(
                s8[0:1].rearrange("o b -> o b ()"),
                cs[0:1, :].rearrange("o (b t) -> o b t", t=NT),
                axis=mybir.AxisListType.X, op=alu.add)
            r8 = pp.tile([1, BS], f32)
            nc.vector.reciprocal(r8[:], s8[:])
            rbt = pp.tile([1, 128], f32)
            nc.vector.tensor_copy(
                rbt[0:1].rearrange("o (b t) -> o b t", t=NT),
                r8[0:1].rearrange("o b -> o b ()").broadcast_to([1, BS, NT]))
            # K=1 matmul: rcol[(b,t), 0] = rbt[(b,t)]
            rcol = psp.tile([128, 1], f32, tag="ps")
            nc.tensor.matmul(rcol[:], rbt[:], ones1[0:1, :])
            rcs = pp.tile([128, 1], f32)
            nc.vector.tensor_copy(rcs[:], rcol[:])

            # transpose P to [(b,t), p]; apply 1/sum as a per-partition ACT
            # scale on the PSUM->SBUF copy; store directly
            PT = psp.tile([128, 128], f32, tag="ps")
            nc.tensor.transpose(PT[:], P[:], ident[:])
            osb = pp.tile([128, 128], f32)
            nc.scalar.activation(osb[:], PT[:], actf.Copy, scale=rcs[:, 0:1])
            nc.sync.dma_start(
                out.rearrange("b o (t p) -> (b o t) p", p=128), osb[:])

    nc.compile()
    return nc


def _get_nc():
    if "nc" not in _CACHE:
        _CACHE["nc"] = _build_nc()
    return _CACHE["nc"]


def kernel(hidden, encoder_outputs, embedding, affect_matrix):
    from concourse.bass_utils import run_bass_kernel_spmd

    nc = _get_nc()
    hidden = np.asarray(hidden, dtype=np.float32)
    encoder_outputs = np.asarray(encoder_outputs, dtype=np.float32)
    embedding = np.asarray(embedding, dtype=np.float32)
    affect_matrix = np.asarray(affect_matrix, dtype=np.float32)

    in_maps = []
    for c in range(N_CORES):
        sl = slice(c * BS, (c + 1) * BS)
        in_maps.append({
            "enc": np.ascontiguousarray(encoder_outputs[:, sl, :]),
            "emb": np.ascontiguousarray(embedding[:, sl, :]),
            "hid": np.ascontiguousarray(hidden[:, sl, :]),
            "amat": affect_matrix,
        })
    res = run_bass_kernel_spmd(nc, in_maps, list(range(N_CORES)))
    return np.concatenate([res.results[c]["out"] for c in range(N_CORES)], axis=0)



# revision 15
# speedup vs baseline: 1.0351x; 1.0351x over previous
"""Luong attention energies + softmax on 8 TRN2 NeuronCores.

reference math (per core, batch-sharded):
  energy[b,s] = <hid[b], enc[s,b]> + (hid[b] @ A) . emb[s,b]
  out[b,0,s]  = softmax_s(energy[b,s])

Full shapes: hidden [1,64,512] f32, encoder_outputs [2048,64,512] f32,
embedding [2048,64,3] f32, affect_matrix [512,3] f32 -> out [64,1,2048] f32.

Sharding: batch dim 64 -> 8 cores x 8. No cross-core communication.

Per-core plan (memory-bound: 32 MB encoder shard; HBM-per-NC ~358 GB/s
=> ~94 us stream floor; the stream itself already runs at that rate, so
the wins are startup and tail):
  stream per s-tile (2 MB DMA, bufs=6 lookahead):
    DVE : one grouped mult (all 8 b) -> pd, reduce b0-1 (+ b2 on
          every 4th tile); last two tiles shift batches to DVE so both
          engines finish together.
    ACT : Copy-with-accum reduces the rest, junk out in PSUM.
  startup: hid broadcast to 128 partitions via two gpsimd
  partition_broadcasts off a 16KB hidrow DMA (scalar HWDGE ring, lands
  ~7.5us); tile 0's DMA and mult are split b0-3/b4-7 so compute starts
  as soon as the first chunk + hidbA land. The affect chain (hA = hid@A
  off a host-replicated A, aff = sum_e emb*hA off a host-pre-transposed
  emb) runs on the otherwise-idle DVE before tile 0 data arrives.
  No SWDGE DMAs anywhere.
  epilogue: one aff add (transposed view), exp(e/2-25) on ACT then
  squared on DVE (= exp(e-50), keeps the ACT exp LUT in range); PE
  ones-matmul column sums; DVE reciprocal; PE transpose puts (b,t) on
  partitions so the 1/sum is a per-partition ACT scale fused into the
  PSUM->SBUF copy; direct strided store.
"""

import numpy as np

S, B, H, E = 2048, 64, 512, 3
N_CORES = 8
BS = B // N_CORES      # 8 batches per core
NT = S // 128          # 16 s-tiles of 128 rows

_CACHE = {}


def _build_nc():
    import concourse.bass as bass
    import concourse.tile as tile
    from concourse import bacc, mybir
    from concourse.mybir import AluOpType as alu
    from concourse.mybir import ActivationFunctionType as actf

    f32 = mybir.dt.float32

    nc = bacc.Bacc("TRN2", target_bir_lowering=False, debug=False)
    enc = nc.dram_tensor("enc", [S, BS, H], f32, kind="ExternalInput").ap()
    embT = nc.dram_tensor("embT", [128, NT * BS * E], f32, kind="ExternalInput").ap()
    hid = nc.dram_tensor("hid", [1, BS * H], f32, kind="ExternalInput").ap()
    a8 = nc.dram_tensor("a8", [BS, H * E], f32, kind="ExternalInput").ap()
    out = nc.dram_tensor("out", [BS, 1, S], f32, kind="ExternalOutput").ap()

    with tile.TileContext(nc) as tc:
        with (
            tc.tile_pool(name="persist", bufs=1) as pp,
            tc.tile_pool(name="enc", bufs=5) as encp,
            tc.tile_pool(name="pd", bufs=4) as pdp,
            tc.tile_pool(name="pjunk", bufs=1, space="PSUM") as pjp,
            tc.tile_pool(name="psum", bufs=2, space="PSUM") as psp,
        ):
            # ---- prologue DMAs: all small loads on the scalar HWDGE ring
            # (enc stream owns the sync ring) ----
            hidrow = pp.tile([1, BS * H], f32)
            nc.scalar.dma_start(hidrow[:], hid)
            hid8 = pp.tile([BS, H], f32)
            nc.scalar.dma_start(hid8[:], hid.rearrange("o (b h) -> (o b) h", h=H))
            a8s = pp.tile([BS, H * E], f32)
            nc.scalar.dma_start(a8s[:], a8)
            embs = pp.tile([128, NT * BS * E], f32)
            nc.scalar.dma_start(embs[:], embT)

            # ---- gpsimd queue: iotas (identity inputs) first, then the
            # hidb broadcast in two chunks so b0-3 unblock early ----
            pidx = pp.tile([128, 1], f32)
            nc.gpsimd.iota(pidx[:], pattern=[[0, 1]], base=0,
                           channel_multiplier=1,
                           allow_small_or_imprecise_dtypes=True)
            colidx = pp.tile([128, 128], f32)
            nc.gpsimd.iota(colidx[:], pattern=[[1, 128]], base=0,
                           channel_multiplier=0,
                           allow_small_or_imprecise_dtypes=True)
            hidbA = pp.tile([128, 4 * H], f32)   # b0..3
            hidbB = pp.tile([128, 4 * H], f32)   # b4..7
            nc.gpsimd.partition_broadcast(hidbA[:], hidrow[0:1, 0:4 * H])
            nc.gpsimd.partition_broadcast(hidbB[:], hidrow[0:1, 4 * H:8 * H])

            # ---- DVE constants ----
            onesr = pp.tile([1, 128], f32)
            nc.vector.memset(onesr[:], 1.0)
            ones1 = pp.tile([128, 1], f32)
            nc.vector.memset(ones1[:], 1.0)
            ebias = pp.tile([128, 1], f32)
            nc.vector.memset(ebias[:], -25.0)
            ident = pp.tile([128, 128], f32)
            nc.vector.tensor_scalar(ident[:], colidx[:], pidx[:, 0:1],
                                    None, alu.is_equal)

            # ---- affect chain on idle DVE before tile 0 lands ----
            # hA[b,e] = sum_h hid8[b,h] * A[h,e]
            hA = pp.tile([BS, E], f32)
            hAtmp = pp.tile([BS, H], f32)
            a8v = a8s[:].rearrange("b (h e) -> b h e", e=E)
            for e in range(E):
                nc.vector.tensor_tensor(hAtmp[:], hid8[:], a8v[:, :, e],
                                        alu.mult)
                nc.vector.tensor_reduce(
                    hA[:, e:e + 1].rearrange("b e -> b e ()"),
                    hAtmp[:].rearrange("b h -> b () h"),
                    axis=mybir.AxisListType.X, op=alu.add)
            # hab[p, (b,e)] = hA[b,e] on all partitions: K=1 PE matmul
            harow = pp.tile([1, BS * E], f32)
            nc.scalar.dma_start(
                harow[0:1].rearrange("o (b e) -> o b e", e=E), hA[:])
            hab_ps = psp.tile([128, BS * E], f32, tag="ps")
            nc.tensor.matmul(hab_ps[:], onesr[:], harow[:])
            hab = pp.tile([128, BS * E], f32)
            nc.scalar.copy(hab[:], hab_ps[:])
            # aff[p, (t,b)] = sum_e emb[t*128+p, b, e] * hA[b, e]
            embs_v = embs[:].rearrange("p (t b e) -> p t b e", b=BS, e=E)
            afftmp = pp.tile([128, NT * BS * E], f32)
            nc.vector.tensor_tensor(
                afftmp[:].rearrange("p (t b e) -> p t b e", b=BS, e=E),
                embs_v,
                hab[:].rearrange("p (b e) -> p b e", e=E)
                .unsqueeze(1).broadcast_to([128, NT, BS, E]),
                alu.mult)
            aff = pp.tile([128, NT * BS], f32)
            nc.vector.tensor_reduce(
                aff[:].rearrange("p (t b) -> p t b", b=BS),
                afftmp[:].rearrange("p (t b e) -> p t b e", b=BS, e=E),
                axis=mybir.AxisListType.X, op=alu.add)

            # ---- main loop ----
            Ebuf = pp.tile([128, 128], f32)      # col = b*NT + t
            E_v = Ebuf[:].rearrange("p (b t) -> p b t", t=NT)
            junkA = pjp.tile([128, H], f32)      # ACT accum main-out (PSUM)

            def dve_reduce(pd_v, t, b0, b1):
                nc.vector.tensor_reduce(
                    E_v[:, b0:b1, t:t + 1], pd_v[:, b0:b1, :],
                    axis=mybir.AxisListType.X, op=alu.add)

            def act_reduce(pd_v, t, b):
                nc.scalar.activation(
                    junkA[:], pd_v[:, b, :], actf.Copy,
                    accum_out=Ebuf[:, b * NT + t:b * NT + t + 1])

            for t in range(NT):
                et = encp.tile([128, BS * H], f32, tag="et")
                et_v = et[:].rearrange("p (b h) -> p b h", h=H)
                pd = pdp.tile([128, BS * H], f32, tag="pd")
                pd_v = pd[:].rearrange("p (b h) -> p b h", h=H)
                if t == 0:
                    # split DMA + mult so b0-3 compute starts as soon as
                    # the first half + hidbA land
                    nc.sync.dma_start(et_v[:, 0:4, :], enc[0:128, 0:4, :])
                    nc.sync.dma_start(et_v[:, 4:8, :], enc[0:128, 4:8, :])
                    nc.vector.tensor_tensor(
                        pd_v[:, 0:4, :], et_v[:, 0:4, :],
                        hidbA[:].rearrange("p (b h) -> p b h", h=H), alu.mult)
                    dve_reduce(pd_v, t, 0, 2)
                    nc.vector.tensor_tensor(
                        pd_v[:, 4:8, :], et_v[:, 4:8, :],
                        hidbB[:].rearrange("p (b h) -> p b h", h=H), alu.mult)
                else:
                    nc.sync.dma_start(et_v, enc[t * 128:(t + 1) * 128])
                    hidb_v = (hidbA[:].rearrange("p (b h) -> p b h", h=H),
                              hidbB[:].rearrange("p (b h) -> p b h", h=H))
                    nc.vector.tensor_tensor(pd_v[:, 0:4, :], et_v[:, 0:4, :],
                                            hidb_v[0], alu.mult)
                    nc.vector.tensor_tensor(pd_v[:, 4:8, :], et_v[:, 4:8, :],
                                            hidb_v[1], alu.mult)
                    dve_reduce(pd_v, t, 0, 2)

                if t == NT - 1:      # rebalance: DVE b0-4, ACT b5-7
                    dve_reduce(pd_v, t, 2, 5)
                    for b in range(5, BS):
                        act_reduce(pd_v, t, b)
                elif t == NT - 2:    # DVE b0-2, ACT b3-7
                    dve_reduce(pd_v, t, 2, 3)
                    for b in range(3, BS):
                        act_reduce(pd_v, t, b)
                else:
                    if t % 4 == 0:
                        dve_reduce(pd_v, t, 2, 3)
                    else:
                        act_reduce(pd_v, t, 2)
                    for b in range(3, BS):
                        act_reduce(pd_v, t, b)

            # ---- epilogue ----
            # add the affect term (transposed view), exp(e/2-25), square
            nc.vector.tensor_tensor(
                E_v, E_v, aff[:].rearrange("p (t b) -> p t b", b=BS)
                .transpose([0, 2, 1]), alu.add)
            P = pp.tile([128, 128], f32)
            nc.scalar.activation(P[:], Ebuf[:], actf.Exp,
                                 bias=ebias[:, 0:1], scale=0.5)
            nc.vector.tensor_tensor(P[:], P[:], P[:], alu.mult)

            # transpose P to [(b,t), p] while the sum chain runs
            PT = psp.tile([128, 128], f32, tag="ps")
            nc.tensor.transpose(PT[:], P[:], ident[:])
            # column sums over the 128 s-partitions: cs[0, b*16+t]
            cs = psp.tile([128, 128], f32, tag="ps")
            nc.tensor.matmul(cs[0:1, :], ones1[:], P[:])
            s8 = pp.tile([1, BS], f32)
            nc.vector.tensor_reduce(
                s8[0:1].rearrange("o b -> o b ()"),
                cs[0:1, :].rearrange("o (b t) -> o b t", t=NT),
                axis=mybir.AxisListType.X, op=alu.add)
            r8 = pp.tile([1, BS], f32)
            nc.vector.reciprocal(r8[:], s8[:])
            rbt = pp.tile([1, 128], f32)
            nc.vector.tensor_copy(
                rbt[0:1].rearrange("o (b t) -> o b t", t=NT),
                r8[0:1].rearrange("o b -> o b ()").broadcast_to([1, BS, NT]))
            # K=1 matmul: rcol[(b,t), 0] = rbt[(b,t)]
            rcol = psp.tile([128, 1], f32, tag="ps")
            nc.tensor.matmul(rcol[:], rbt[:], ones1[0:1, :])
            rcs = pp.tile([128, 1], f32)
            nc.vector.tensor_copy(rcs[:], rcol[:])
            # apply 1/sum as a per-partition ACT scale on the PSUM->SBUF copy
            osb = pp.tile([128, 128], f32)
            nc.scalar.activation(osb[:], PT[:], actf.Copy, scale=rcs[:, 0:1])
            nc.sync.dma_start(
                out.rearrange("b o (t p) -> (b o t) p", p=128), osb[:])

    nc.compile()
    return nc


def _get_nc():
    if "nc" not in _CACHE:
        _CACHE["nc"] = _build_nc()
    return _CACHE["nc"]


def _make_in_maps(hidden, encoder_outputs, embedding, affect_matrix):
    a8 = np.ascontiguousarray(
        np.broadcast_to(affect_matrix.reshape(1, H * E), (BS, H * E)))
    in_maps = []
    for c in range(N_CORES):
        sl = slice(c * BS, (c + 1) * BS)
        embc = embedding[:, sl, :]  # [S, BS, E]
        in_maps.append({
            "enc": np.ascontiguousarray(encoder_outputs[:, sl, :]),
            "embT": np.ascontiguousarray(
                embc.reshape(NT, 128, BS, E).transpose(1, 0, 2, 3)
                .reshape(128, NT * BS * E)),
            "hid": np.ascontiguousarray(hidden[:, sl, :].reshape(1, BS * H)),
            "a8": a8,
        })
    return in_maps


def kernel(hidden, encoder_outputs, embedding, affect_matrix):
    from concourse.bass_utils import run_bass_kernel_spmd

    nc = _get_nc()
    hidden = np.asarray(hidden, dtype=np.float32)
    encoder_outputs = np.asarray(encoder_outputs, dtype=np.float32)
    embedding = np.asarray(embedding, dtype=np.float32)
    affect_matrix = np.asarray(affect_matrix, dtype=np.float32)

    in_maps = _make_in_maps(hidden, encoder_outputs, embedding, affect_matrix)
    res = run_bass_kernel_spmd(nc, in_maps, list(range(N_CORES)))
    return np.concatenate([res.results[c]["out"] for c in range(N_CORES)], axis=0)


# revision 19
# speedup vs baseline: 1.0930x; 1.0558x over previous
"""Luong attention energies + softmax on 8 TRN2 NeuronCores.

reference math (per core, batch-sharded):
  energy[b,s] = <hid[b], enc[s,b]> + (hid[b] @ A) . emb[s,b]
  out[b,0,s]  = softmax_s(energy[b,s])

Full shapes: hidden [1,64,512] f32, encoder_outputs [2048,64,512] f32,
embedding [2048,64,3] f32, affect_matrix [512,3] f32 -> out [64,1,2048] f32.

Sharding: batch dim 64 -> 8 cores x 8. No cross-core communication.

Per-core plan (memory-bound: 32 MB encoder shard; HBM-per-NC ~358 GB/s
=> ~94 us stream floor; the stream itself already runs at that rate, so
the wins are startup and tail):
  stream per s-tile (2 MB DMA, bufs=6 lookahead):
    DVE : one grouped mult (all 8 b) -> pd, reduce b0-1 (+ b2 on
          every 4th tile); last two tiles shift batches to DVE so both
          engines finish together.
    ACT : Copy-with-accum reduces the rest, junk out in PSUM.
  startup: hid broadcast to 128 partitions via two gpsimd
  partition_broadcasts off a 16KB hidrow DMA (scalar HWDGE ring, lands
  ~7.5us); tile 0's DMA and mult are split b0-3/b4-7 so compute starts
  as soon as the first chunk + hidbA land. The affect chain (hA = hid@A
  off a host-replicated A, aff = sum_e emb*hA off a host-pre-transposed
  emb) runs on the otherwise-idle DVE before tile 0 data arrives.
  No SWDGE DMAs anywhere.
  epilogue: one aff add (transposed view), exp(e/2-25) on ACT then
  squared on DVE (= exp(e-50), keeps the ACT exp LUT in range); PE
  ones-matmul column sums; DVE reciprocal; PE transpose puts (b,t) on
  partitions so the 1/sum is a per-partition ACT scale fused into the
  PSUM->SBUF copy; direct strided store.
"""

import numpy as np

S, B, H, E = 2048, 64, 512, 3
N_CORES = 8
BS = B // N_CORES      # 8 batches per core
NT = S // 128          # 16 s-tiles of 128 rows

_CACHE = {}


def _build_nc():
    import concourse.bass as bass
    import concourse.tile as tile
    from concourse import bacc, mybir
    from concourse.mybir import AluOpType as alu
    from concourse.mybir import ActivationFunctionType as actf

    f32 = mybir.dt.float32

    nc = bacc.Bacc("TRN2", target_bir_lowering=False, debug=False)
    enc = nc.dram_tensor("enc", [S, BS, H], f32, kind="ExternalInput").ap()
    embT = nc.dram_tensor("embT", [128, NT * BS * E], f32, kind="ExternalInput").ap()
    hid = nc.dram_tensor("hid", [1, BS * H], f32, kind="ExternalInput").ap()
    a8 = nc.dram_tensor("a8", [BS, H * E], f32, kind="ExternalInput").ap()
    out = nc.dram_tensor("out", [BS, 1, S], f32, kind="ExternalOutput").ap()

    with tile.TileContext(nc) as tc:
        with (
            tc.tile_pool(name="persist", bufs=1) as pp,
            tc.tile_pool(name="enc", bufs=5) as encp,
            tc.tile_pool(name="pd", bufs=4) as pdp,
            tc.tile_pool(name="pjunk", bufs=1, space="PSUM") as pjp,
            tc.tile_pool(name="psum", bufs=2, space="PSUM") as psp,
            tc.tile_pool(name="pbc", bufs=4, space="PSUM") as pbcp,
        ):
            # ---- prologue DMAs: small loads FIRST on the sync ring so
            # they land before the enc stream monopolizes the SDMA
            # engines (on the scalar ring they complete ~10us late) ----
            hidrow = pp.tile([1, BS * H], f32)
            nc.sync.dma_start(hidrow[:], hid)
            hid8 = pp.tile([BS, H], f32)
            nc.sync.dma_start(hid8[:], hid.rearrange("o (b h) -> (o b) h", h=H))
            a8s = pp.tile([BS, H * E], f32)
            nc.sync.dma_start(a8s[:], a8)
            embs = pp.tile([128, NT * BS * E], f32)
            nc.sync.dma_start(embs[:], embT)

            # ---- gpsimd queue: iotas (identity inputs) first, then the
            # hidbA broadcast; hidbB comes from PE+ACT in parallel ----
            pidx = pp.tile([128, 1], f32)
            nc.gpsimd.iota(pidx[:], pattern=[[0, 1]], base=0,
                           channel_multiplier=1,
                           allow_small_or_imprecise_dtypes=True)
            colidx = pp.tile([128, 128], f32)
            nc.gpsimd.iota(colidx[:], pattern=[[1, 128]], base=0,
                           channel_multiplier=0,
                           allow_small_or_imprecise_dtypes=True)
            hidbA = pp.tile([128, 4 * H], f32)   # b0..3 (gpsimd bcast)
            hidbB = pp.tile([128, 4 * H], f32)   # b4..7 (PE + ACT drains)
            nc.gpsimd.partition_broadcast(hidbA[:], hidrow[0:1, 0:4 * H])

            # ---- DVE constants ----
            onesr = pp.tile([1, 128], f32)
            nc.vector.memset(onesr[:], 1.0)
            for c in range(4):
                pb = pbcp.tile([128, 512], f32, tag="pb")
                nc.tensor.matmul(pb[:], onesr[:],
                                 hidrow[0:1, (4 + c) * H:(5 + c) * H])
                nc.scalar.copy(hidbB[:, c * H:(c + 1) * H], pb[:])
            ones1 = pp.tile([128, 1], f32)
            nc.vector.memset(ones1[:], 1.0)
            ebias = pp.tile([128, 1], f32)
            nc.vector.memset(ebias[:], -25.0)
            ident = pp.tile([128, 128], f32)
            nc.vector.tensor_scalar(ident[:], colidx[:], pidx[:, 0:1],
                                    None, alu.is_equal)

            # ---- affect chain on idle DVE before tile 0 lands ----
            # hA[b,e] = sum_h hid8[b,h] * A[h,e]
            hA = pp.tile([BS, E], f32)
            hAtmp = pp.tile([BS, H], f32)
            a8v = a8s[:].rearrange("b (h e) -> b h e", e=E)
            for e in range(E):
                nc.vector.tensor_tensor(hAtmp[:], hid8[:], a8v[:, :, e],
                                        alu.mult)
                nc.vector.tensor_reduce(
                    hA[:, e:e + 1].rearrange("b e -> b e ()"),
                    hAtmp[:].rearrange("b h -> b () h"),
                    axis=mybir.AxisListType.X, op=alu.add)
            # hab[p, (b,e)] = hA[b,e] on all partitions: K=1 PE matmul
            harow = pp.tile([1, BS * E], f32)
            nc.scalar.dma_start(
                harow[0:1].rearrange("o (b e) -> o b e", e=E), hA[:])
            hab_ps = psp.tile([128, BS * E], f32, tag="ps")
            nc.tensor.matmul(hab_ps[:], onesr[:], harow[:])
            hab = pp.tile([128, BS * E], f32)
            nc.scalar.copy(hab[:], hab_ps[:])
            # aff[p, (t,b)] = sum_e emb[t*128+p, b, e] * hA[b, e]
            embs_v = embs[:].rearrange("p (t b e) -> p t b e", b=BS, e=E)
            afftmp = pp.tile([128, NT * BS * E], f32)
            nc.vector.tensor_tensor(
                afftmp[:].rearrange("p (t b e) -> p t b e", b=BS, e=E),
                embs_v,
                hab[:].rearrange("p (b e) -> p b e", e=E)
                .unsqueeze(1).broadcast_to([128, NT, BS, E]),
                alu.mult)
            aff = pp.tile([128, NT * BS], f32)
            nc.vector.tensor_reduce(
                aff[:].rearrange("p (t b) -> p t b", b=BS),
                afftmp[:].rearrange("p (t b e) -> p t b e", b=BS, e=E),
                axis=mybir.AxisListType.X, op=alu.add)

            # ---- main loop ----
            Ebuf = pp.tile([128, 128], f32)      # col = b*NT + t
            E_v = Ebuf[:].rearrange("p (b t) -> p b t", t=NT)
            junkA = pjp.tile([128, H], f32)      # ACT accum main-out (PSUM)

            def dve_reduce(pd_v, t, b0, b1):
                nc.vector.tensor_reduce(
                    E_v[:, b0:b1, t:t + 1], pd_v[:, b0:b1, :],
                    axis=mybir.AxisListType.X, op=alu.add)

            def act_reduce(pd_v, t, b):
                nc.scalar.activation(
                    junkA[:], pd_v[:, b, :], actf.Copy,
                    accum_out=Ebuf[:, b * NT + t:b * NT + t + 1])

            for t in range(NT):
                et = encp.tile([128, BS * H], f32, tag="et")
                et_v = et[:].rearrange("p (b h) -> p b h", h=H)
                pd = pdp.tile([128, BS * H], f32, tag="pd")
                pd_v = pd[:].rearrange("p (b h) -> p b h", h=H)
                if t == 0:
                    # split DMA + mult so b0-3 compute starts as soon as
                    # the first half + hidbA land
                    nc.sync.dma_start(et_v[:, 0:4, :], enc[0:128, 0:4, :])
                    nc.sync.dma_start(et_v[:, 4:8, :], enc[0:128, 4:8, :])
                    nc.vector.tensor_tensor(
                        pd_v[:, 0:4, :], et_v[:, 0:4, :],
                        hidbA[:].rearrange("p (b h) -> p b h", h=H), alu.mult)
                    dve_reduce(pd_v, t, 0, 2)
                    nc.vector.tensor_tensor(
                        pd_v[:, 4:8, :], et_v[:, 4:8, :],
                        hidbB[:].rearrange("p (b h) -> p b h", h=H), alu.mult)
                else:
                    nc.sync.dma_start(et_v, enc[t * 128:(t + 1) * 128])
                    hidb_v = (hidbA[:].rearrange("p (b h) -> p b h", h=H),
                              hidbB[:].rearrange("p (b h) -> p b h", h=H))
                    nc.vector.tensor_tensor(pd_v[:, 0:4, :], et_v[:, 0:4, :],
                                            hidb_v[0], alu.mult)
                    nc.vector.tensor_tensor(pd_v[:, 4:8, :], et_v[:, 4:8, :],
                                            hidb_v[1], alu.mult)
                    dve_reduce(pd_v, t, 0, 2)

                if t == NT - 1:      # rebalance: DVE b0-4, ACT b5-7
                    dve_reduce(pd_v, t, 2, 5)
                    for b in range(5, BS):
                        act_reduce(pd_v, t, b)
                elif t == NT - 2:    # DVE b0-2, ACT b3-7
                    dve_reduce(pd_v, t, 2, 3)
                    for b in range(3, BS):
                        act_reduce(pd_v, t, b)
                else:
                    for b in range(2, BS):
                        act_reduce(pd_v, t, b)

            # ---- epilogue ----
            # add the affect term (transposed view), exp(e/2-25), square
            nc.vector.tensor_tensor(
                E_v, E_v, aff[:].rearrange("p (t b) -> p t b", b=BS)
                .transpose([0, 2, 1]), alu.add)
            P = pp.tile([128, 128], f32)
            nc.scalar.activation(P[:], Ebuf[:], actf.Exp,
                                 bias=ebias[:, 0:1], scale=0.5)
            nc.vector.tensor_tensor(P[:], P[:], P[:], alu.mult)

            # transpose P to [(b,t), p] while the sum chain runs
            PT = psp.tile([128, 128], f32, tag="ps")
            nc.tensor.transpose(PT[:], P[:], ident[:])
            # column sums over the 128 s-partitions: cs[0, b*16+t]
            cs = psp.tile([128, 128], f32, tag="ps")
            nc.tensor.matmul(cs[0:1, :], ones1[:], P[:])
            s8 = pp.tile([1, BS], f32)
            nc.vector.tensor_reduce(
                s8[0:1].rearrange("o b -> o b ()"),
                cs[0:1, :].rearrange("o (b t) -> o b t", t=NT),
                axis=mybir.AxisListType.X, op=alu.add)
            r8 = pp.tile([1, BS], f32)
            nc.vector.reciprocal(r8[:], s8[:])
            rbt = pp.tile([1, 128], f32)
            nc.vector.tensor_copy(
                rbt[0:1].rearrange("o (b t) -> o b t", t=NT),
                r8[0:1].rearrange("o b -> o b ()").broadcast_to([1, BS, NT]))
            # K=1 matmul: rcol[(b,t), 0] = rbt[(b,t)]
            rcol = psp.tile([128, 1], f32, tag="ps")
            nc.tensor.matmul(rcol[:], rbt[:], ones1[0:1, :])
            rcs = pp.tile([128, 1], f32)
            nc.vector.tensor_copy(rcs[:], rcol[:])
            # apply 1/sum as a per-partition ACT scale on the PSUM->SBUF copy
            osb = pp.tile([128, 128], f32)
            nc.scalar.activation(osb[:], PT[:], actf.Copy, scale=rcs[:, 0:1])
            nc.sync.dma_start(
                out.rearrange("b o (t p) -> (b o t) p", p=128), osb[:])

    nc.compile()
    return nc


def _get_nc():
    if "nc" not in _CACHE:
        _CACHE["nc"] = _build_nc()
    return _CACHE["nc"]


def _make_in_maps(hidden, encoder_outputs, embedding, affect_matrix):
    a8 = np.ascontiguousarray(
        np.broadcast_to(affect_matrix.reshape(1, H * E), (BS, H * E)))
    in_maps = []
    for c in range(N_CORES):
        sl = slice(c * BS, (c + 1) * BS)
        embc = embedding[:, sl, :]  # [S, BS, E]
        in_maps.append({
            "enc": np.ascontiguousarray(encoder_outputs[:, sl, :]),
            "embT": np.ascontiguousarray(
                embc.reshape(NT, 128, BS, E).transpose(1, 0, 2, 3)
                .reshape(128, NT * BS * E)),
            "hid": np.ascontiguousarray(hidden[:, sl, :].reshape(1, BS * H)),
            "a8": a8,
        })
    return in_maps


def kernel(hidden, encoder_outputs, embedding, affect_matrix):
    from concourse.bass_utils import run_bass_kernel_spmd

    nc = _get_nc()
    hidden = np.asarray(hidden, dtype=np.float32)
    encoder_outputs = np.asarray(encoder_outputs, dtype=np.float32)
    embedding = np.asarray(embedding, dtype=np.float32)
    affect_matrix = np.asarray(affect_matrix, dtype=np.float32)

    in_maps = _make_in_maps(hidden, encoder_outputs, embedding, affect_matrix)
    res = run_bass_kernel_spmd(nc, in_maps, list(range(N_CORES)))
    return np.concatenate([res.results[c]["out"] for c in range(N_CORES)], axis=0)


# revision 25
# speedup vs baseline: 1.1012x; 1.0076x over previous
"""Luong attention energies + softmax on 8 TRN2 NeuronCores.

reference math (per core, batch-sharded):
  energy[b,s] = <hid[b], enc[s,b]> + (hid[b] @ A) . emb[s,b]
  out[b,0,s]  = softmax_s(energy[b,s])

Full shapes: hidden [1,64,512] f32, encoder_outputs [2048,64,512] f32,
embedding [2048,64,3] f32, affect_matrix [512,3] f32 -> out [64,1,2048] f32.

Sharding: batch dim 64 -> 8 cores x 8. No cross-core communication.

Per-core plan (memory-bound: 32 MB encoder shard; HBM-per-NC ~358 GB/s
=> ~94 us stream floor; the stream runs at the SDMA engines' aggregate
rate, so the wins are startup, runway, and tail):
  stream per s-tile (2 MB DMA, bufs=7 lookahead so the issue-side
  semaphore round-trip never throttles the SDMA queues):
    DVE : two half mults (b0-3, b4-7; separate hidb tiles so tile 0
          starts as soon as the first half is broadcast) -> pd,
          reduce b0-1; last two tiles shift batches to DVE so both
          engines finish together.
    ACT : Copy-with-accum reduces b2-7, junk out in PSUM.
  Every 4 tiles, two tiles behind the stream (so no cross-engine
  stall): DVE adds the affect columns, ACT exps them (e/2-25, keeps
  the exp LUT in range), DVE squares. Only the last group remains
  after the stream.
  startup: hid broadcast to 128 partitions via 8 K=1 PE matmuls
  (ones[1,128] x hid8-row -> PSUM) drained by the otherwise-idle ACT;
  no gpsimd partition_broadcast (its Q7 launch alone costs ~6 us). The
  affect chain (hA = hid@A off a host-replicated A, aff = sum_e emb*hA
  off a host-pre-transposed emb) runs on the otherwise-idle DVE before
  tile 0 data arrives. No SWDGE DMAs anywhere. Small loads go FIRST on
  the sync ring: on the scalar ring they complete ~10 us late once the
  enc stream saturates the SDMA engines.
  epilogue: PE ones-matmul column sums; DVE reciprocal; K=1 matmul
  puts 1/sum on (b,t) partitions; PE transpose of the probabilities;
  ACT applies the per-partition 1/sum scale on the PSUM->SBUF copy;
  direct strided store.
"""

import numpy as np

S, B, H, E = 2048, 64, 512, 3
N_CORES = 8
BS = B // N_CORES      # 8 batches per core
NT = S // 128          # 16 s-tiles of 128 rows

_CACHE = {}


def _build_nc():
    import concourse.bass as bass
    import concourse.tile as tile
    from concourse import bacc, mybir
    from concourse.mybir import AluOpType as alu
    from concourse.mybir import ActivationFunctionType as actf

    f32 = mybir.dt.float32

    nc = bacc.Bacc("TRN2", target_bir_lowering=False, debug=False)
    enc = nc.dram_tensor("enc", [S, BS, H], f32, kind="ExternalInput").ap()
    embT = nc.dram_tensor("embT", [128, NT * BS * E], f32, kind="ExternalInput").ap()
    hid = nc.dram_tensor("hid", [1, BS * H], f32, kind="ExternalInput").ap()
    amat = nc.dram_tensor("amat", [H, E], f32, kind="ExternalInput").ap()
    out = nc.dram_tensor("out", [BS, 1, S], f32, kind="ExternalOutput").ap()

    with tile.TileContext(nc) as tc:
        with (
            tc.tile_pool(name="persist", bufs=1) as pp,
            tc.tile_pool(name="enc", bufs=7) as encp,
            tc.tile_pool(name="pd", bufs=3) as pdp,
            tc.tile_pool(name="pjunk", bufs=1, space="PSUM") as pjp,
            tc.tile_pool(name="psum", bufs=2, space="PSUM") as psp,
            tc.tile_pool(name="pbc", bufs=4, space="PSUM") as pbcp,
        ):
            # ---- prologue DMAs: small loads FIRST on the sync ring ----
            hidrow = pp.tile([1, BS * H], f32)
            nc.sync.dma_start(hidrow[:], hid)
            hid8 = pp.tile([BS, H], f32)
            nc.sync.dma_start(hid8[:], hid.rearrange("o (b h) -> (o b) h", h=H))
            am128 = pp.tile([128, 4 * E], f32)
            nc.sync.dma_start(
                am128[:].rearrange("p (c e) -> p c e", e=E),
                amat.rearrange("(c p) e -> p c e", p=128))
            embs = pp.tile([128, NT * BS * E], f32)
            nc.sync.dma_start(embs[:], embT)

            # ---- gpsimd: iotas for the identity matrix ----
            pidx = pp.tile([128, 1], f32)
            nc.gpsimd.iota(pidx[:], pattern=[[0, 1]], base=0,
                           channel_multiplier=1,
                           allow_small_or_imprecise_dtypes=True)
            colidx = pp.tile([128, 128], f32)
            nc.gpsimd.iota(colidx[:], pattern=[[1, 128]], base=0,
                           channel_multiplier=0,
                           allow_small_or_imprecise_dtypes=True)

            # ---- DVE constants ----
            onesr = pp.tile([1, 128], f32)
            nc.vector.memset(onesr[:], 1.0)
            ones1 = pp.tile([128, 1], f32)
            nc.vector.memset(ones1[:], 1.0)
            ebias = pp.tile([128, 1], f32)
            nc.vector.memset(ebias[:], -25.0)
            ident = pp.tile([128, 128], f32)
            nc.vector.tensor_scalar(ident[:], colidx[:], pidx[:, 0:1],
                                    None, alu.is_equal)

            # ---- hid broadcast: K=1 PE matmul per 512-chunk, reading
            # hid8 rows directly; idle ACT drains PSUM->SBUF ----
            hidbA = pp.tile([128, 4 * H], f32)   # b0..3
            hidbB = pp.tile([128, 4 * H], f32)   # b4..7
            for c in range(8):
                pb = pbcp.tile([128, 512], f32, tag="pb")
                nc.tensor.matmul(pb[:], onesr[:],
                                 hidrow[0:1, c * H:(c + 1) * H])
                dst = hidbA if c < 4 else hidbB
                nc.scalar.copy(dst[:, (c % 4) * H:(c % 4 + 1) * H], pb[:])

            # ---- affect chain before tile 0 lands (PE + idle DVE) ----
            # hA[b,e] = sum_h hid[b,h] * A[h,e]: transpose hid8 into
            # [h', (c, b)] chunks, then 4 accumulating [128,8]x[128,3]
            # matmuls against the DRAM-partitioned A
            hT_ps = psp.tile([128, 4 * BS], f32, tag="ps")
            for c in range(4):
                nc.tensor.transpose(
                    hT_ps[:, c * BS:(c + 1) * BS],
                    hid8[:, c * 128:(c + 1) * 128], ident[0:BS, 0:BS])
            hT = pp.tile([128, 4 * BS], f32)
            nc.vector.tensor_copy(hT[:], hT_ps[:])
            hA_ps = psp.tile([BS, E], f32, tag="ps")
            for c in range(4):
                nc.tensor.matmul(
                    hA_ps[:], hT[:, c * BS:(c + 1) * BS],
                    am128[:].rearrange("p (c e) -> p c e", e=E)[:, c, :],
                    start=(c == 0), stop=(c == 3))
            hA = pp.tile([BS, E], f32)
            nc.vector.tensor_copy(hA[:], hA_ps[:])
            # hab[p, (b,e)] = hA[b,e] on all partitions: K=1 PE matmul
            harow = pp.tile([1, BS * E], f32)
            nc.scalar.dma_start(
                harow[0:1].rearrange("o (b e) -> o b e", e=E), hA[:])
            hab_ps = psp.tile([128, BS * E], f32, tag="ps")
            nc.tensor.matmul(hab_ps[:], onesr[:], harow[:])
            hab = pp.tile([128, BS * E], f32)
            nc.scalar.copy(hab[:], hab_ps[:])
            # aff[p, (t,b)] = sum_e emb[t*128+p, b, e] * hA[b, e]
            embs_v = embs[:].rearrange("p (t b e) -> p t b e", b=BS, e=E)
            afftmp = pp.tile([128, NT * BS * E], f32)
            nc.vector.tensor_tensor(
                afftmp[:].rearrange("p (t b e) -> p t b e", b=BS, e=E),
                embs_v,
                hab[:].rearrange("p (b e) -> p b e", e=E)
                .unsqueeze(1).broadcast_to([128, NT, BS, E]),
                alu.mult)
            aff = pp.tile([128, NT * BS], f32)
            nc.vector.tensor_reduce(
                aff[:].rearrange("p (t b) -> p t b", b=BS),
                afftmp[:].rearrange("p (t b e) -> p t b e", b=BS, e=E),
                axis=mybir.AxisListType.X, op=alu.add)
            aff_tb = aff[:].rearrange("p (t b) -> p t b", b=BS)

            # ---- main loop ----
            Ebuf = pp.tile([128, 128], f32)      # col = b*NT + t
            E_v = Ebuf[:].rearrange("p (b t) -> p b t", t=NT)
            P = pp.tile([128, 128], f32)         # exp(E/2 - 25), squared
            P_v = P[:].rearrange("p (b t) -> p b t", t=NT)
            junkA = pjp.tile([128, H], f32)      # ACT accum main-out (PSUM)
            hA_v = (hidbA[:].rearrange("p (b h) -> p b h", h=H),
                    hidbB[:].rearrange("p (b h) -> p b h", h=H))

            def dve_reduce(pd_v, t, b0, b1):
                nc.vector.tensor_reduce(
                    E_v[:, b0:b1, t:t + 1], pd_v[:, b0:b1, :],
                    axis=mybir.AxisListType.X, op=alu.add)

            def act_reduce(pd_v, t, b):
                nc.scalar.activation(
                    junkA[:], pd_v[:, b, :], actf.Copy,
                    accum_out=Ebuf[:, b * NT + t:b * NT + t + 1])

            def tail_group(g):
                # energies of tiles 4g..4g+3 are final: +aff, exp, square
                sl = slice(4 * g, 4 * g + 4)
                nc.vector.tensor_tensor(
                    E_v[:, :, sl], E_v[:, :, sl],
                    aff_tb[:, sl, :].transpose([0, 2, 1]), alu.add)
                nc.scalar.activation(P_v[:, :, sl], E_v[:, :, sl], actf.Exp,
                                     bias=ebias[:, 0:1], scale=0.5)
                nc.vector.tensor_tensor(P_v[:, :, sl], P_v[:, :, sl],
                                        P_v[:, :, sl], alu.mult)

            for t in range(NT):
                et = encp.tile([128, BS * H], f32, tag="et")
                et_v = et[:].rearrange("p (b h) -> p b h", h=H)
                pd = pdp.tile([128, BS * H], f32, tag="pd")
                pd_v = pd[:].rearrange("p (b h) -> p b h", h=H)
                if t == 0:
                    # split DMA so b0-3 compute starts as the first
                    # half + hidbA land
                    nc.sync.dma_start(et_v[:, 0:4, :], enc[0:128, 0:4, :])
                    nc.sync.dma_start(et_v[:, 4:8, :], enc[0:128, 4:8, :])
                else:
                    nc.sync.dma_start(et_v, enc[t * 128:(t + 1) * 128])
                nc.vector.tensor_tensor(pd_v[:, 0:4, :], et_v[:, 0:4, :],
                                        hA_v[0], alu.mult)
                dve_reduce(pd_v, t, 0, 2)
                nc.vector.tensor_tensor(pd_v[:, 4:8, :], et_v[:, 4:8, :],
                                        hA_v[1], alu.mult)

                if t == NT - 1:      # rebalance: DVE b0-4, ACT b5-7
                    dve_reduce(pd_v, t, 2, 5)
                    for b in range(5, BS):
                        act_reduce(pd_v, t, b)
                elif t == NT - 2:    # DVE b0-2, ACT b3-7
                    dve_reduce(pd_v, t, 2, 3)
                    for b in range(3, BS):
                        act_reduce(pd_v, t, b)
                else:
                    for b in range(2, BS):
                        act_reduce(pd_v, t, b)
                # finalize an earlier 4-tile group, 2 tiles behind so the
                # cross-engine deps (ACT accums) are already settled
                if t >= 5 and (t - 5) % 4 == 0:
                    tail_group((t - 5) // 4)

            tail_group(3)

            # ---- epilogue ----
            # column sums over the 128 s-partitions: cs[0, b*16+t]
            cs = psp.tile([128, 128], f32, tag="ps")
            nc.tensor.matmul(cs[0:1, :], ones1[:], P[:])
            # transpose P to [(b,t), p] while the sum chain runs
            PT = psp.tile([128, 128], f32, tag="ps")
            nc.tensor.transpose(PT[:], P[:], ident[:])
            s8 = pp.tile([1, BS], f32)
            nc.vector.tensor_reduce(
                s8[0:1].rearrange("o b -> o b ()"),
                cs[0:1, :].rearrange("o (b t) -> o b t", t=NT),
                axis=mybir.AxisListType.X, op=alu.add)
            r8 = pp.tile([1, BS], f32)
            nc.vector.reciprocal(r8[:], s8[:])
            rbt = pp.tile([1, 128], f32)
            nc.vector.tensor_copy(
                rbt[0:1].rearrange("o (b t) -> o b t", t=NT),
                r8[0:1].rearrange("o b -> o b ()").broadcast_to([1, BS, NT]))
            # K=1 matmul: rcol[(b,t), 0] = rbt[(b,t)]
            rcol = psp.tile([128, 1], f32, tag="ps")
            nc.tensor.matmul(rcol[:], rbt[:], ones1[0:1, :])
            rcs = pp.tile([128, 1], f32)
            nc.vector.tensor_copy(rcs[:], rcol[:])
            # apply 1/sum as a per-partition ACT scale on the PSUM->SBUF copy
            osb = pp.tile([128, 128], f32)
            nc.scalar.activation(osb[:], PT[:], actf.Copy, scale=rcs[:, 0:1])
            nc.sync.dma_start(
                out.rearrange("b o (t p) -> (b o t) p", p=128), osb[:])

    nc.compile()
    return nc


def _get_nc():
    if "nc" not in _CACHE:
        _CACHE["nc"] = _build_nc()
    return _CACHE["nc"]


def _make_in_maps(hidden, encoder_outputs, embedding, affect_matrix):
    in_maps = []
    for c in range(N_CORES):
        sl = slice(c * BS, (c + 1) * BS)
        embc = embedding[:, sl, :]  # [S, BS, E]
        in_maps.append({
            "enc": np.ascontiguousarray(encoder_outputs[:, sl, :]),
            "embT": np.ascontiguousarray(
                embc.reshape(NT, 128, BS, E).transpose(1, 0, 2, 3)
                .reshape(128, NT * BS * E)),
            "hid": np.ascontiguousarray(hidden[:, sl, :].reshape(1, BS * H)),
            "amat": np.ascontiguousarray(affect_matrix),
        })
    return in_maps


def kernel(hidden, encoder_outputs, embedding, affect_matrix):
    from concourse.bass_utils import run_bass_kernel_spmd

    nc = _get_nc()
    hidden = np.asarray(hidden, dtype=np.float32)
    encoder_outputs = np.asarray(encoder_outputs, dtype=np.float32)
    embedding = np.asarray(embedding, dtype=np.float32)
    affect_matrix = np.asarray(affect_matrix, dtype=np.float32)

    in_maps = _make_in_maps(hidden, encoder_outputs, embedding, affect_matrix)
    res = run_bass_kernel_spmd(nc, in_maps, list(range(N_CORES)))
    return np.concatenate([res.results[c]["out"] for c in range(N_CORES)], axis=0)


# revision 32
# speedup vs baseline: 1.1835x; 1.0747x over previous
"""Luong attention energies + softmax on 8 TRN2 NeuronCores.

reference math (per core, batch-sharded):
  energy[b,s] = <hid[b], enc[s,b]> + (hid[b] @ A) . emb[s,b]
  out[b,0,s]  = softmax_s(energy[b,s])

Full shapes: hidden [1,64,512] f32, encoder_outputs [2048,64,512] f32,
embedding [2048,64,3] f32, affect_matrix [512,3] f32 -> out [64,1,2048] f32.

Sharding: batch dim 64 -> 8 cores x 8. No cross-core communication.

Per-core plan (memory-bound: 32 MB encoder shard; HBM-per-NC ~358 GB/s
=> ~94 us stream floor; the stream runs at the SDMA engines' aggregate
rate, so the wins are startup, runway, and tail):
  stream per s-tile (2 MB DMA, bufs=7 lookahead so the issue-side
  semaphore round-trip never throttles the SDMA queues):
    DVE : two half mults (b0-3, b4-7; separate hidb tiles so tile 0
          starts as soon as the first half is broadcast) -> pd,
          reduce b0-1; last two tiles shift batches to DVE so both
          engines finish together.
    ACT : Copy-with-accum reduces b2-7, junk out in PSUM.
  Every 4 tiles, two tiles behind the stream (so no cross-engine
  stall): DVE adds the affect columns, ACT exps them (e/2-25, keeps
  the exp LUT in range), DVE squares. Only the last group remains
  after the stream.
  startup: hid broadcast to 128 partitions via 8 K=1 PE matmuls
  (ones[1,128] x hid8-row -> PSUM) drained by the otherwise-idle ACT;
  no gpsimd partition_broadcast (its Q7 launch alone costs ~6 us). The
  affect chain (hA = hid@A off a host-replicated A, aff = sum_e emb*hA
  off a host-pre-transposed emb) runs on the otherwise-idle DVE before
  tile 0 data arrives. No SWDGE DMAs anywhere. Small loads go FIRST on
  the sync ring: on the scalar ring they complete ~10 us late once the
  enc stream saturates the SDMA engines.
  epilogue: PE ones-matmul column sums; DVE reciprocal; K=1 matmul
  puts 1/sum on (b,t) partitions; PE transpose of the probabilities;
  ACT applies the per-partition 1/sum scale on the PSUM->SBUF copy;
  direct strided store.
"""

import numpy as np

S, B, H, E = 2048, 64, 512, 3
N_CORES = 8
BS = B // N_CORES      # 8 batches per core
NT = S // 128          # 16 s-tiles of 128 rows

_CACHE = {}


def _build_nc():
    import concourse.bass as bass
    import concourse.tile as tile
    from concourse import bacc, mybir
    from concourse.mybir import AluOpType as alu
    from concourse.mybir import ActivationFunctionType as actf

    f32 = mybir.dt.float32

    bf16 = mybir.dt.bfloat16

    nc = bacc.Bacc("TRN2", target_bir_lowering=False, debug=False)
    enc = nc.dram_tensor("enc", [S, BS, H], f32, kind="ExternalInput").ap()
    embT = nc.dram_tensor("embT", [128, NT * BS * E], f32, kind="ExternalInput").ap()
    hid = nc.dram_tensor("hid", [1, BS * H], f32, kind="ExternalInput").ap()
    hid2 = nc.dram_tensor("hid2", [2, BS * H], bf16, kind="ExternalInput").ap()
    amat = nc.dram_tensor("amat", [H, E], f32, kind="ExternalInput").ap()
    out = nc.dram_tensor("out", [BS, 1, S], f32, kind="ExternalOutput").ap()

    with tile.TileContext(nc) as tc:
        with (
            tc.tile_pool(name="persist", bufs=1) as pp,
            tc.tile_pool(name="enc", bufs=7) as encp,
            tc.tile_pool(name="pd", bufs=3) as pdp,
            tc.tile_pool(name="pjunk", bufs=1, space="PSUM") as pjp,
            tc.tile_pool(name="psum", bufs=2, space="PSUM") as psp,
            tc.tile_pool(name="pbc", bufs=4, space="PSUM") as pbcp,
        ):
            # ---- prologue DMAs: small loads FIRST on the sync ring ----
            # hid2: exact bf16 hi/lo split of hid (host-prepped) so the
            # broadcast matmuls run the PE at bf16 rate (f32 moving
            # operands are ~4x slower); K=2 ones rebuilds hi+lo in f32
            hid2s = pp.tile([2, BS * H], bf16)
            nc.sync.dma_start(hid2s[:], hid2)
            hid8 = pp.tile([BS, H], f32)
            nc.sync.dma_start(hid8[:], hid.rearrange("o (b h) -> (o b) h", h=H))
            am128 = pp.tile([128, 4 * E], f32)
            nc.sync.dma_start(
                am128[:].rearrange("p (c e) -> p c e", e=E),
                amat.rearrange("(c p) e -> p c e", p=128))
            embs = pp.tile([128, NT * BS * E], f32)
            nc.sync.dma_start(embs[:], embT)

            # ---- gpsimd: iotas for the identity matrix ----
            pidx = pp.tile([128, 1], f32)
            nc.gpsimd.iota(pidx[:], pattern=[[0, 1]], base=0,
                           channel_multiplier=1,
                           allow_small_or_imprecise_dtypes=True)
            colidx = pp.tile([128, 128], f32)
            nc.gpsimd.iota(colidx[:], pattern=[[1, 128]], base=0,
                           channel_multiplier=0,
                           allow_small_or_imprecise_dtypes=True)

            # ---- DVE constants ----
            onesr = pp.tile([1, 128], f32)
            nc.vector.memset(onesr[:], 1.0)
            ones2 = pp.tile([2, 128], bf16)
            nc.vector.memset(ones2[:], 1.0)
            ones1 = pp.tile([128, 1], f32)
            nc.vector.memset(ones1[:], 1.0)
            ebias = pp.tile([128, 1], f32)
            nc.vector.memset(ebias[:], -25.0)
            ident = pp.tile([128, 128], f32)
            nc.vector.tensor_scalar(ident[:], colidx[:], pidx[:, 0:1],
                                    None, alu.is_equal)

            # ---- affect chain part 1, PE first (it must clear the PE
            # before the broadcast matmuls own it):
            # hA[b,e] = sum_h hid[b,h] * A[h,e]: transpose hid8 into
            # [h', (c, b)] chunks, then 4 accumulating [128,8]x[128,3]
            # matmuls against the DRAM-partitioned A
            hT_ps = psp.tile([128, 4 * BS], f32, tag="ps")
            for c in range(4):
                nc.tensor.transpose(
                    hT_ps[:, c * BS:(c + 1) * BS],
                    hid8[:, c * 128:(c + 1) * 128], ident[0:BS, 0:BS])
            hT = pp.tile([128, 4 * BS], f32)
            nc.vector.tensor_copy(hT[:], hT_ps[:])
            hA_ps = psp.tile([BS, E], f32, tag="ps")
            for c in range(4):
                nc.tensor.matmul(
                    hA_ps[:], hT[:, c * BS:(c + 1) * BS],
                    am128[:].rearrange("p (c e) -> p c e", e=E)[:, c, :],
                    start=(c == 0), stop=(c == 3))
            hA = pp.tile([BS, E], f32)
            nc.vector.tensor_copy(hA[:], hA_ps[:])
            harow = pp.tile([1, BS * E], f32)
            nc.scalar.dma_start(
                harow[0:1].rearrange("o (b e) -> o b e", e=E), hA[:])

            # ---- hid broadcast: one K=2 bf16 matmul per 512-chunk
            # (hi+lo accumulate to f32 hid in PSUM); ACT drains ----
            hidbA = pp.tile([128, 4 * H], f32)   # b0..3
            hidbB = pp.tile([128, 4 * H], f32)   # b4..7
            hab_ps = psp.tile([128, BS * E], f32, tag="ps")
            hab = pp.tile([128, BS * E], f32)
            for c in range(8):
                pb = pbcp.tile([128, 512], f32, tag="pb")
                nc.tensor.matmul(pb[:], ones2[:],
                                 hid2s[:, c * H:(c + 1) * H])
                dst = hidbA if c < 4 else hidbB
                nc.scalar.copy(dst[:, (c % 4) * H:(c % 4 + 1) * H], pb[:])
                if c == 3:
                    # hab[p, (b,e)] = hA[b,e] on all partitions, slotted
                    # between the A and B halves (harow has landed)
                    nc.tensor.matmul(hab_ps[:], onesr[:], harow[:])
                    nc.scalar.copy(hab[:], hab_ps[:])

            embs_v = embs[:].rearrange("p (t b e) -> p t b e", b=BS, e=E)
            afftmp = pp.tile([128, NT * BS * E], f32)
            aff = pp.tile([128, NT * BS], f32)
            aff_tb = aff[:].rearrange("p (t b) -> p t b", b=BS)

            def emit_aff():
                # aff[p, (t,b)] = sum_e emb[t*128+p, b, e] * hA[b, e];
                # runs in the DVE window while waiting for hidbB
                nc.vector.tensor_tensor(
                    afftmp[:].rearrange("p (t b e) -> p t b e", b=BS, e=E),
                    embs_v,
                    hab[:].rearrange("p (b e) -> p b e", e=E)
                    .unsqueeze(1).broadcast_to([128, NT, BS, E]),
                    alu.mult)
                nc.vector.tensor_reduce(
                    aff[:].rearrange("p (t b) -> p t b", b=BS),
                    afftmp[:].rearrange("p (t b e) -> p t b e", b=BS, e=E),
                    axis=mybir.AxisListType.X, op=alu.add)

            # ---- main loop ----
            Ebuf = pp.tile([128, 128], f32)      # col = b*NT + t
            E_v = Ebuf[:].rearrange("p (b t) -> p b t", t=NT)
            P = pp.tile([128, 128], f32)         # exp(E/2 - 25), squared
            P_v = P[:].rearrange("p (b t) -> p b t", t=NT)
            junkA = pjp.tile([128, H], f32)      # ACT accum main-out (PSUM)
            hA_v = (hidbA[:].rearrange("p (b h) -> p b h", h=H),
                    hidbB[:].rearrange("p (b h) -> p b h", h=H))

            def dve_reduce(pd_v, t, b0, b1):
                nc.vector.tensor_reduce(
                    E_v[:, b0:b1, t:t + 1], pd_v[:, b0:b1, :],
                    axis=mybir.AxisListType.X, op=alu.add)

            def act_reduce(pd_v, t, b):
                nc.scalar.activation(
                    junkA[:], pd_v[:, b, :], actf.Copy,
                    accum_out=Ebuf[:, b * NT + t:b * NT + t + 1])

            def tail_group(g):
                # energies of tiles 4g..4g+3 are final: +aff, exp, square
                sl = slice(4 * g, 4 * g + 4)
                nc.vector.tensor_tensor(
                    E_v[:, :, sl], E_v[:, :, sl],
                    aff_tb[:, sl, :].transpose([0, 2, 1]), alu.add)
                nc.scalar.activation(P_v[:, :, sl], E_v[:, :, sl], actf.Exp,
                                     bias=ebias[:, 0:1], scale=0.5)
                nc.vector.tensor_tensor(P_v[:, :, sl], P_v[:, :, sl],
                                        P_v[:, :, sl], alu.mult)

            for t in range(NT):
                et = encp.tile([128, BS * H], f32, tag="et")
                et_v = et[:].rearrange("p (b h) -> p b h", h=H)
                pd = pdp.tile([128, BS * H], f32, tag="pd")
                pd_v = pd[:].rearrange("p (b h) -> p b h", h=H)
                if t == 0:
                    # split DMA so b0-3 compute starts as the first
                    # half + hidbA land
                    nc.sync.dma_start(et_v[:, 0:4, :], enc[0:128, 0:4, :])
                    nc.sync.dma_start(et_v[:, 4:8, :], enc[0:128, 4:8, :])
                else:
                    nc.sync.dma_start(et_v, enc[t * 128:(t + 1) * 128])
                nc.vector.tensor_tensor(pd_v[:, 0:4, :], et_v[:, 0:4, :],
                                        hA_v[0], alu.mult)
                dve_reduce(pd_v, t, 0, 2)
                if t == 0:
                    emit_aff()
                nc.vector.tensor_tensor(pd_v[:, 4:8, :], et_v[:, 4:8, :],
                                        hA_v[1], alu.mult)

                if t == NT - 1:      # rebalance: DVE b0-4, ACT b5-7
                    dve_reduce(pd_v, t, 2, 5)
                    for b in range(5, BS):
                        act_reduce(pd_v, t, b)
                elif t == NT - 2:    # DVE b0-2, ACT b3-7
                    dve_reduce(pd_v, t, 2, 3)
                    for b in range(3, BS):
                        act_reduce(pd_v, t, b)
                else:
                    for b in range(2, BS):
                        act_reduce(pd_v, t, b)
                # finalize an earlier 4-tile group, 2 tiles behind so the
                # cross-engine deps (ACT accums) are already settled
                if t >= 5 and (t - 5) % 4 == 0:
                    tail_group((t - 5) // 4)

            tail_group(3)

            # ---- epilogue ----
            # column sums over the 128 s-partitions: cs[0, b*16+t]
            cs = psp.tile([128, 128], f32, tag="ps")
            nc.tensor.matmul(cs[0:1, :], ones1[:], P[:])
            # transpose P to [(b,t), p] while the sum chain runs
            PT = psp.tile([128, 128], f32, tag="ps")
            nc.tensor.transpose(PT[:], P[:], ident[:])
            s8 = pp.tile([1, BS], f32)
            nc.vector.tensor_reduce(
                s8[0:1].rearrange("o b -> o b ()"),
                cs[0:1, :].rearrange("o (b t) -> o b t", t=NT),
                axis=mybir.AxisListType.X, op=alu.add)
            r8 = pp.tile([1, BS], f32)
            nc.vector.reciprocal(r8[:], s8[:])
            rbt = pp.tile([1, 128], f32)
            nc.vector.tensor_copy(
                rbt[0:1].rearrange("o (b t) -> o b t", t=NT),
                r8[0:1].rearrange("o b -> o b ()").broadcast_to([1, BS, NT]))
            # K=1 matmul: rcol[(b,t), 0] = rbt[(b,t)]
            rcol = psp.tile([128, 1], f32, tag="ps")
            nc.tensor.matmul(rcol[:], rbt[:], ones1[0:1, :])
            rcs = pp.tile([128, 1], f32)
            nc.vector.tensor_copy(rcs[:], rcol[:])
            # apply 1/sum as a per-partition ACT scale on the PSUM->SBUF copy
            osb = pp.tile([128, 128], f32)
            nc.scalar.activation(osb[:], PT[:], actf.Copy, scale=rcs[:, 0:1])
            nc.sync.dma_start(
                out.rearrange("b o (t p) -> (b o t) p", p=128), osb[:])

    nc.compile()
    return nc


def _get_nc():
    if "nc" not in _CACHE:
        _CACHE["nc"] = _build_nc()
    return _CACHE["nc"]


def _make_in_maps(hidden, encoder_outputs, embedding, affect_matrix):
    import ml_dtypes
    bf16 = ml_dtypes.bfloat16
    in_maps = []
    for c in range(N_CORES):
        sl = slice(c * BS, (c + 1) * BS)
        embc = embedding[:, sl, :]  # [S, BS, E]
        hrow = hidden[:, sl, :].reshape(1, BS * H).astype(np.float32)
        hi = hrow.astype(bf16)
        lo = (hrow - hi.astype(np.float32)).astype(bf16)
        in_maps.append({
            "enc": np.ascontiguousarray(encoder_outputs[:, sl, :]),
            "embT": np.ascontiguousarray(
                embc.reshape(NT, 128, BS, E).transpose(1, 0, 2, 3)
                .reshape(128, NT * BS * E)),
            "hid": hrow,
            "hid2": np.ascontiguousarray(np.concatenate([hi, lo], axis=0)),
            "amat": np.ascontiguousarray(affect_matrix),
        })
    return in_maps


def kernel(hidden, encoder_outputs, embedding, affect_matrix):
    from concourse.bass_utils import run_bass_kernel_spmd

    nc = _get_nc()
    hidden = np.asarray(hidden, dtype=np.float32)
    encoder_outputs = np.asarray(encoder_outputs, dtype=np.float32)
    embedding = np.asarray(embedding, dtype=np.float32)
    affect_matrix = np.asarray(affect_matrix, dtype=np.float32)

    in_maps = _make_in_maps(hidden, encoder_outputs, embedding, affect_matrix)
    res = run_bass_kernel_spmd(nc, in_maps, list(range(N_CORES)))
    return np.concatenate([res.results[c]["out"] for c in range(N_CORES)], axis=0)
